# revision 23
# baseline (speedup 1.0000x reference)
"""Trainium2 Bass kernel for nn_Decoder_TRANSFORMER_14791867367496.

The reference decoder is affine in the positions: each frame step is
    pos_{t+1} = pos_t @ M + (d_t[b] + g[b,j]),   M = I + W_pe @ W3  (3x3)
(with W_final = [W1; W2; W3] split along its 768 input rows), so the whole
60-step scan has a closed form

    out[b, j, :, t] = X[b, j, :] @ Q_t + r_t[b, :]

where X = initial_grid,
    Q_t = M^t + (W_pe @ W2) @ S_t,          S_t = sum_{k<t} M^k
    r_t[b] = h @ S_t + D_t[b],              D_t = sum_{s=1..t} d_s M^{t-s}
    d_t[b] = (emb_table[t] + z @ W_clip + b_clip) @ W1
    h      = b_pe @ (W2 + W3) + b_final

All of Q/r are tiny (3x3 / per-batch 3-vectors) and are computed on the host
in float64.  The device kernel is then a single affine map per point
([3 feats + bias] -> 180 outputs) and is purely output-bandwidth bound.

The whole device pipeline runs in fp16: the correctness gate is an L2
relative error of 2e-2 and fp16 operands + fp16 output storage land at
~2.9e-4, so the kernel streams the output as fp16 (half the HBM bytes of
f32 — the per-core HBM limit ~358 GB/s is the roofline) and the host
unshard step upcasts to f32.

Device structure (per core: 4 batches = 16384 points = 128 point-tiles):
 - 64 matmuls, each covering a pair of point-tiles ([K=8, 128] stationary
   x [8, 360] block-diagonal rhs -> [128, 360] PSUM).  Sequential MMs at
   one tile position keep the LDWEIGHTS double-buffer path correct (a
   concurrent 4-position row-tiled variant measurably corrupts the
   streaming matmul's weights).  A dense back-to-back MM stream also
   un-throttles the PE HAM clock gate (1.2 -> 2.4 GHz) ~3.4 us in.
 - One [128, 4096] f32 PSUM tensor = all 8 banks; matmul j writes the
   512-col-aligned slot j%8.
 - PSUM->SBUF fp16-converting copies run 1 elem/cycle (PSUM source keeps
   DVE/ACT at 1x mode), so per-instruction fixed cost is amortized with
   wide strided copies: one copy per 4 matmuls reads 4 slots (FD=1440).
   DVE takes even units, ACT odd units; units 0/1 are split finer so the
   output stream starts right after matmul 0.
 - Output: 4 groups x 1.47 MB (per-DMA efficiency ~341 GB/s at this
   size); group 0 goes out as 1/16,1/16,1/8,1/4,1/2.  3 stage buffers
   decouple copies from DMA.  Odd groups issue on the ACT HWDGE ring,
   the rest on SP, so per-DMA setup bubbles overlap.
"""

import numpy as np

BS, NFRAMES, NJOINTS, NFEATS, LATENT, CLIP = 32, 60, 4096, 3, 256, 512
NCORES = 8
B_PER_CORE = BS // NCORES                  # 4
PTS = B_PER_CORE * NJOINTS                 # 16384 points per core
NTILES = PTS // 128                        # 128 point-tiles per core
GROUPS = 4                                 # output DMA groups
TPG = NTILES // GROUPS                     # 32 tiles per group
FC = NFEATS * NFRAMES                      # 180 output columns per point
KR = 4                                     # K rows per tile (3 feats + bias)
PAIR = 2                                   # tiles fused per matmul
NMM = NTILES // PAIR                       # 64 matmuls per core
NUNIT = NMM // 4                           # 16 copy units (4 MMs each)
XC = NMM * 128                             # xt columns (8192)
SLOT = 512                                 # psum cols per matmul slot (bank)
UPG = NUNIT // GROUPS                      # 4 copy units per group


def _precompute(z, W_pe, b_pe, W_clip, b_clip, emb_table, W_final, b_final):
    """Host-side f64 computation of the closed-form coefficients.

    Returns Q_all [3, 180] and r_all [32, 180], column layout c = f*60 + t
    (matching the [.., 3, 60] innermost layout of the output)."""
    f64 = np.float64
    W_pe64 = np.asarray(W_pe, f64)
    W_fin = np.asarray(W_final, f64)
    W1, W2, W3 = W_fin[:LATENT], W_fin[LATENT:2 * LATENT], W_fin[2 * LATENT:]
    M = np.eye(3) + W_pe64 @ W3
    Gm = W_pe64 @ W2
    b_pe64 = np.asarray(b_pe, f64)
    h = b_pe64 @ W2 + b_pe64 @ W3 + np.asarray(b_final, f64)
    z_proj = np.asarray(z, f64) @ np.asarray(W_clip, f64) + np.asarray(b_clip, f64)
    d = (np.asarray(emb_table, f64)[None, :, :] + z_proj[:, None, :]) @ W1  # [32,60,3]

    Q = np.zeros((NFRAMES, 3, 3))
    R = np.zeros((NFRAMES, BS, 3))
    Q[0] = np.eye(3)
    Mt = np.eye(3)
    S = np.zeros((3, 3))
    D = np.zeros((BS, 3))
    for t in range(1, NFRAMES):
        S = S + Mt
        Mt = Mt @ M
        D = D @ M + d[:, t, :]
        Q[t] = Mt + Gm @ S
        R[t] = h @ S + D
    Q_all = Q.transpose(1, 2, 0).reshape(3, FC)     # [k, f*60+t]
    r_all = R.transpose(1, 2, 0).reshape(BS, FC)    # [b, f*60+t]
    return Q_all.astype(np.float32), r_all.astype(np.float32)


N_STAGE = 3   # stage buffers

# copy plan: unit u (= matmuls 4u..4u+3, psum slots (u%2)*4..+3) ->
# list of (engine, first_slot, n_slots) pieces.  Unit 0 is split fine so
# the first output DMAs can start early; from unit 1 on, two FD=720
# pieces per unit (DVE even units, ACT odd) — the first piece starts two
# matmuls early, which removes the PE psum-wrap stall entirely.
_COPY_PLAN = {0: [("v", 0, 1), ("a", 1, 1), ("v", 2, 2)]}
for _u in range(1, NUNIT):
    _e = "v" if _u % 2 == 0 else "a"
    _p = (_u % 2) * 4
    _COPY_PLAN[_u] = [(_e, _p, 2), (_e, _p + 2, 2)]
# the final unit is split across both engines so the stream tail drains
# as early as possible
_COPY_PLAN[NUNIT - 1] = [("v", 4, 2), ("a", 6, 2)]

# engine-stream position (1-based) of each copy piece, in emission order
_COPY_POS = {}
_cnt = {"v": 0, "a": 0}
for _u in range(NUNIT):
    for _i, (_e, _s0, _ns) in enumerate(_COPY_PLAN[_u]):
        _cnt[_e] += 1
        _COPY_POS[(_u, _i)] = (_e, _cnt[_e])


def _unit_done_pos(u):
    """Per-engine copy positions after which every slot of unit u is
    drained -> dict engine -> min required count."""
    need = {}
    for i, (e, _s0, _ns) in enumerate(_COPY_PLAN[u]):
        _e, pos = _COPY_POS[(u, i)]
        need[_e] = max(need.get(_e, 0), pos)
    return need


def _group_need(g):
    """Copy positions needed before group g's stage buffer is full."""
    need = {}
    for u in range(UPG * g, UPG * (g + 1)):
        for e, pos in _unit_done_pos(u).items():
            need[e] = max(need.get(e, 0), pos)
    return need


def _build_bass():
    import concourse.mybir as mybir
    from concourse import bacc
    from concourse.bass import ts

    f32 = mybir.dt.float32
    f16 = mybir.dt.float16
    nc = bacc.Bacc(None, target_bir_lowering=False)
    xt = nc.dram_tensor("xt", [PAIR * KR, XC], f16, kind="ExternalInput")
    rhs = nc.dram_tensor("rhs", [PAIR * KR, B_PER_CORE * PAIR * FC], f16,
                         kind="ExternalInput")
    out = nc.dram_tensor("out", [PTS, FC], f16, kind="ExternalOutput")
    out_v = out[:].rearrange("(g j w) c -> g j (w c)", g=GROUPS, j=128, w=TPG)
    out_v4 = out[:].rearrange("(g j w) c -> g j w c", g=GROUPS, j=128, w=TPG)

    from contextlib import ExitStack
    ctx = ExitStack()
    rhs_sb = ctx.enter_context(
        nc.sbuf_tensor("rhs_sb", [PAIR * KR, B_PER_CORE * PAIR * FC], f16))
    xt_sb = ctx.enter_context(nc.sbuf_tensor("xt_sb", [PAIR * KR, XC], f16))
    stage = [ctx.enter_context(
        nc.sbuf_tensor(f"stage{i}", [128, TPG * FC], f16))
        for i in range(N_STAGE)]
    ps = ctx.enter_context(nc.psum_tensor("ps", [128, 8 * SLOT], f32))
    s_rhs = ctx.enter_context(nc.semaphore("s_rhs"))
    s_x = ctx.enter_context(nc.semaphore("s_x"))
    s_pe = ctx.enter_context(nc.semaphore("s_pe"))
    s_cpv = ctx.enter_context(nc.semaphore("s_cpv"))
    s_cpa = ctx.enter_context(nc.semaphore("s_cpa"))
    s_slot = [ctx.enter_context(nc.semaphore(f"s_slot{i}"))
              for i in range(N_STAGE)]

    # ---- input DMAs (one per tensor — completion latency dominates, so
    # fewer receipts beat smaller chunks): xt 128 KB on SP, rhs 23 KB on
    # the ACT ring ----
    nc.sync.dma_start(out=xt_sb[:], in_=xt[:]).then_inc(s_x, 16)
    nc.scalar.dma_start(out=rhs_sb[:], in_=rhs[:]).then_inc(s_rhs, 16)

    # out-DMA incs previously issued on each stage slot, for reuse waits
    prior_dmas = {3: 5}

    # ---- PE: 64 back-to-back matmuls ----
    # (The PE HAM clock gate never opens in this environment — a 3.4 us
    # dummy warm-up burst was measured to leave the stream at 1.2 GHz —
    # so the matmuls run at the cold 300 ns/360-col cadence and the copy
    # plan is shaped to never stall the PE.)
    for j in range(NMM):
        lb = j // 16
        u = j // 4
        if j == 0:
            nc.tensor.wait_ge(s_x, 16)
            nc.tensor.wait_ge(s_rhs, 16)
        if j % 4 == 0 and u >= 2:
            # psum half reuse: wait for the copies that drained unit u-2
            for e, pos in _unit_done_pos(u - 2).items():
                nc.tensor.wait_ge(s_cpv if e == "v" else s_cpa, pos)
        slot = j % 8
        nc.tensor.matmul(
            ps[:, slot * SLOT: slot * SLOT + PAIR * FC],
            xt_sb[:, ts(j, 128)],
            rhs_sb[:, ts(lb, PAIR * FC)],
            start=True, stop=True,
        ).then_inc(s_pe, 1)

    # ---- DVE/ACT: PSUM -> fp16 stage copies per _COPY_PLAN ----
    def emit_copies(engine_name):
        engine = nc.vector if engine_name == "v" else nc.scalar
        for u in range(NUNIT):
            g = u // UPG
            first_in_group = (u % UPG) == min(
                uu for uu in range(UPG)
                if any(e == engine_name for e, _s, _n in _COPY_PLAN[g * UPG + uu]))
            for i, (e, s0, ns) in enumerate(_COPY_PLAN[u]):
                if e != engine_name:
                    continue
                if g >= N_STAGE and first_in_group and i == min(
                        ii for ii, (ee, _s, _n) in enumerate(_COPY_PLAN[u])
                        if ee == engine_name):
                    engine.wait_ge(s_slot[g % N_STAGE],
                                   16 * prior_dmas[g])
                engine.wait_ge(s_pe, 4 * u + (s0 - (u % 2) * 4) + ns)
                src = (ps[:, s0 * SLOT:(s0 + ns) * SLOT]
                       .rearrange("p (s c) -> p s c", s=ns, c=SLOT)
                       [:, :, :PAIR * FC])
                base = (u % UPG) * 4 * PAIR * FC
                within = s0 - (u % 2) * 4
                dst = (stage[g % N_STAGE]
                       [:, base + within * PAIR * FC:
                        base + (within + ns) * PAIR * FC]
                       .rearrange("p (s c) -> p s c", s=ns, c=PAIR * FC))
                if engine_name == "v":
                    nc.vector.tensor_copy(out=dst, in_=src).then_inc(s_cpv, 1)
                else:
                    nc.scalar.copy(out=dst, in_=src).then_inc(s_cpa, 1)
            # ACT issues group 1's DMA after its copy that completes the
            # group, and the last group's odd units as per-unit DMAs so
            # the stream tail isn't gated on the whole group
            if engine_name == "a" and g == 1 and u == UPG * 2 - 1:
                need = _group_need(g)
                if need.get("v"):
                    nc.scalar.wait_ge(s_cpv, need["v"])
                nc.scalar.dma_start(
                    out=out_v[g], in_=stage[g % N_STAGE][:]
                ).then_inc(s_slot[g % N_STAGE], 16)
            if engine_name == "a" and g == GROUPS - 1 and u % 2 == 1:
                w0 = (u % UPG) * (TPG // UPG)
                w1 = w0 + TPG // UPG
                if u == NUNIT - 1:
                    w0 += TPG // UPG // 2   # ACT half: slots 6-7 only
                nc.scalar.dma_start(
                    out=out_v4[GROUPS - 1][:, w0:w1, :],
                    in_=stage[(GROUPS - 1) % N_STAGE][:, w0 * FC:w1 * FC],
                ).then_inc(s_slot[(GROUPS - 1) % N_STAGE], 16)

    emit_copies("a")
    emit_copies("v")

    # ---- SP: group 0 split DMAs, group 2, and the last group's even
    # units (per-unit, so the stream tail drains as copies land) ----
    g0_waits = []
    for pieces, w0, w1 in (([(0, 0)], 0, 2), ([(0, 1)], 2, 4),
                           ([(0, 2)], 4, 8), ([(1, 0), (1, 1)], 8, 16),
                           ([(2, 1), (3, 1)], 16, TPG)):
        nv = na = 0
        for up in pieces:
            e, pos = _COPY_POS[up]
            if e == "v":
                nv = max(nv, pos)
            else:
                na = max(na, pos)
        g0_waits.append((nv, na, w0, w1))
    for nv, na, w0, w1 in g0_waits:
        if nv:
            nc.sync.wait_ge(s_cpv, nv)
        if na:
            nc.sync.wait_ge(s_cpa, na)
        nc.sync.dma_start(
            out=out_v4[0][:, w0:w1, :],
            in_=stage[0][:, w0 * FC:w1 * FC],
        ).then_inc(s_slot[0], 16)
    g = 2
    need = _group_need(g)
    if need.get("v"):
        nc.sync.wait_ge(s_cpv, need["v"])
    if need.get("a"):
        nc.sync.wait_ge(s_cpa, need["a"])
    nc.sync.dma_start(out=out_v[g], in_=stage[g % N_STAGE][:]).then_inc(
        s_slot[g % N_STAGE], 16)
    for u in range(UPG * (GROUPS - 1), NUNIT):
        if u % 2 == 1 and u != NUNIT - 1:
            continue      # odd units issue on the ACT ring
        w0 = (u % UPG) * (TPG // UPG)
        w1 = w0 + TPG // UPG
        if u == NUNIT - 1:
            w1 -= TPG // UPG // 2   # SP half: the DVE-copied slots 4-5
            nc.sync.wait_ge(s_cpv, _COPY_POS[(u, 0)][1])
        else:
            nc.sync.wait_ge(s_cpv, _unit_done_pos(u)["v"])
        nc.sync.dma_start(
            out=out_v4[GROUPS - 1][:, w0:w1, :],
            in_=stage[(GROUPS - 1) % N_STAGE][:, w0 * FC:w1 * FC],
        ).then_inc(s_slot[(GROUPS - 1) % N_STAGE], 16)

    ctx.close()
    nc.finalize()
    return nc


_NC_CACHE = None
_LAST_RESULTS = None  # BassKernelResults of the most recent run (for profiling)


def kernel(z, mask, initial_grid, W_pe, b_pe, W_clip, b_clip, emb_table,
           W_final, b_final):
    global _NC_CACHE, _LAST_RESULTS
    from concourse import bass_utils

    Q_all, r_all = _precompute(z, W_pe, b_pe, W_clip, b_clip, emb_table,
                               W_final, b_final)
    X = np.ascontiguousarray(np.asarray(initial_grid), dtype=np.float32)

    in_maps = []
    for c in range(NCORES):
        Xc = X[B_PER_CORE * c:B_PER_CORE * (c + 1)].reshape(PTS, NFEATS)
        # point p = g*4096 + j*32 + w lives at tile (g, w), psum partition j
        X4 = Xc.reshape(GROUPS, 128, TPG, NFEATS).transpose(3, 0, 2, 1)
        A = np.empty((GROUPS, TPG, KR, 128), np.float32)
        for k in range(NFEATS):
            A[:, :, k, :] = X4[k]
        A[:, :, NFEATS, :] = 1.0                        # bias row
        # matmul j covers tiles (2*(j%16), 2*(j%16)+1) of group j//16;
        # stationary rows 4a..4a+3 hold tile a of the pair
        xt_host = (A.reshape(GROUPS, TPG // PAIR, PAIR, KR, 128)
                   .transpose(2, 3, 0, 1, 4)
                   .reshape(PAIR * KR, XC)).astype(np.float16)

        rhs_host = np.zeros((PAIR * KR, B_PER_CORE * PAIR * FC), np.float32)
        for lb in range(B_PER_CORE):
            R = np.empty((KR, FC), np.float32)
            R[:NFEATS] = Q_all
            R[NFEATS] = r_all[B_PER_CORE * c + lb]
            for a in range(PAIR):                       # block-diagonal
                rhs_host[KR * a:KR * (a + 1),
                         lb * PAIR * FC + FC * a: lb * PAIR * FC + FC * (a + 1)] = R
        in_maps.append({"xt": np.ascontiguousarray(xt_host),
                        "rhs": rhs_host.astype(np.float16)})

    if _NC_CACHE is None:
        _NC_CACHE = _build_bass()
    res = bass_utils.run_bass_kernel_spmd(
        _NC_CACHE, in_maps, core_ids=list(range(NCORES))
    )
    _LAST_RESULTS = res

    out = np.empty((BS, NJOINTS, NFEATS, NFRAMES), np.float32)
    for c in range(NCORES):
        out[B_PER_CORE * c:B_PER_CORE * (c + 1)] = (
            np.asarray(res.results[c]["out"], dtype=np.float32)
            .reshape(B_PER_CORE, NJOINTS, NFEATS, NFRAMES)
        )
    return out


# revision 25
# speedup vs baseline: 1.0593x; 1.0593x over previous
"""Trainium2 Bass kernel for nn_Decoder_TRANSFORMER_14791867367496.

The reference decoder is affine in the positions: each frame step is
    pos_{t+1} = pos_t @ M + (d_t[b] + g[b,j]),   M = I + W_pe @ W3  (3x3)
(with W_final = [W1; W2; W3] split along its 768 input rows), so the whole
60-step scan has a closed form

    out[b, j, :, t] = X[b, j, :] @ Q_t + r_t[b, :]

where X = initial_grid,
    Q_t = M^t + (W_pe @ W2) @ S_t,          S_t = sum_{k<t} M^k
    r_t[b] = h @ S_t + D_t[b],              D_t = sum_{s=1..t} d_s M^{t-s}
    d_t[b] = (emb_table[t] + z @ W_clip + b_clip) @ W1
    h      = b_pe @ (W2 + W3) + b_final

All of Q/r are tiny (3x3 / per-batch 3-vectors) and are computed on the host
in float64.  The device kernel is then a single affine map per point
([3 feats + bias] -> 180 outputs) and is purely output-bandwidth bound.

The whole device pipeline runs in fp16: the correctness gate is an L2
relative error of 2e-2 and fp16 operands + fp16 output storage land at
~2.9e-4, so the kernel streams the output as fp16 (half the HBM bytes of
f32 — the per-core HBM limit ~358 GB/s is the roofline) and the host
unshard step upcasts to f32.

Device structure (per core: 4 batches = 16384 points = 128 point-tiles):
 - 64 matmuls, each covering a pair of point-tiles ([K=8, 128] stationary
   x [8, 360] block-diagonal rhs -> [128, 360] PSUM).  Sequential MMs at
   one tile position keep the LDWEIGHTS double-buffer path correct (a
   concurrent 4-position row-tiled variant measurably corrupts the
   streaming matmul's weights).  A dense back-to-back MM stream also
   un-throttles the PE HAM clock gate (1.2 -> 2.4 GHz) ~3.4 us in.
 - One [128, 4096] f32 PSUM tensor = all 8 banks; matmul j writes the
   512-col-aligned slot j%8.
 - PSUM->SBUF fp16-converting copies run 1 elem/cycle (PSUM source keeps
   DVE/ACT at 1x mode), so per-instruction fixed cost is amortized with
   wide strided copies: one copy per 4 matmuls reads 4 slots (FD=1440).
   DVE takes even units, ACT odd units; units 0/1 are split finer so the
   output stream starts right after matmul 0.
 - Output: 4 groups x 1.47 MB (per-DMA efficiency ~341 GB/s at this
   size); group 0 goes out as 1/16,1/16,1/8,1/4,1/2.  3 stage buffers
   decouple copies from DMA.  Odd groups issue on the ACT HWDGE ring,
   the rest on SP, so per-DMA setup bubbles overlap.
"""

import numpy as np

BS, NFRAMES, NJOINTS, NFEATS, LATENT, CLIP = 32, 60, 4096, 3, 256, 512
NCORES = 8
B_PER_CORE = BS // NCORES                  # 4
PTS = B_PER_CORE * NJOINTS                 # 16384 points per core
NTILES = PTS // 128                        # 128 point-tiles per core
GROUPS = 4                                 # output DMA groups
TPG = NTILES // GROUPS                     # 32 tiles per group
FC = NFEATS * NFRAMES                      # 180 output columns per point
KR = 4                                     # K rows per tile (3 feats + bias)
PAIR = 2                                   # tiles fused per matmul
NMM = NTILES // PAIR                       # 64 matmuls per core
NUNIT = NMM // 4                           # 16 copy units (4 MMs each)
XC = NMM * 128                             # xt columns (8192)
SLOT = 512                                 # psum cols per matmul slot (bank)
UPG = NUNIT // GROUPS                      # 4 copy units per group


def _precompute(z, W_pe, b_pe, W_clip, b_clip, emb_table, W_final, b_final):
    """Host-side f64 computation of the closed-form coefficients.

    Returns Q_all [3, 180] and r_all [32, 180], column layout c = f*60 + t
    (matching the [.., 3, 60] innermost layout of the output)."""
    f64 = np.float64
    W_pe64 = np.asarray(W_pe, f64)
    W_fin = np.asarray(W_final, f64)
    W1, W2, W3 = W_fin[:LATENT], W_fin[LATENT:2 * LATENT], W_fin[2 * LATENT:]
    M = np.eye(3) + W_pe64 @ W3
    Gm = W_pe64 @ W2
    b_pe64 = np.asarray(b_pe, f64)
    h = b_pe64 @ W2 + b_pe64 @ W3 + np.asarray(b_final, f64)
    z_proj = np.asarray(z, f64) @ np.asarray(W_clip, f64) + np.asarray(b_clip, f64)
    d = (np.asarray(emb_table, f64)[None, :, :] + z_proj[:, None, :]) @ W1  # [32,60,3]

    Q = np.zeros((NFRAMES, 3, 3))
    R = np.zeros((NFRAMES, BS, 3))
    Q[0] = np.eye(3)
    Mt = np.eye(3)
    S = np.zeros((3, 3))
    D = np.zeros((BS, 3))
    for t in range(1, NFRAMES):
        S = S + Mt
        Mt = Mt @ M
        D = D @ M + d[:, t, :]
        Q[t] = Mt + Gm @ S
        R[t] = h @ S + D
    Q_all = Q.transpose(1, 2, 0).reshape(3, FC)     # [k, f*60+t]
    r_all = R.transpose(1, 2, 0).reshape(BS, FC)    # [b, f*60+t]
    return Q_all.astype(np.float32), r_all.astype(np.float32)


N_STAGE = 3   # stage buffers

# copy plan: unit u (= matmuls 4u..4u+3, psum slots (u%2)*4..+3) ->
# list of (engine, first_slot, n_slots) pieces.  Unit 0 is split fine so
# the first output DMAs can start early; from unit 1 on, two FD=720
# pieces per unit (DVE even units, ACT odd) — the first piece starts two
# matmuls early, which removes the PE psum-wrap stall entirely.
_COPY_PLAN = {0: [("v", 0, 1), ("a", 1, 1), ("v", 2, 2)]}
for _u in range(1, NUNIT):
    _e = "v" if _u % 2 == 0 else "a"
    _p = (_u % 2) * 4
    _COPY_PLAN[_u] = [(_e, _p, 2), (_e, _p + 2, 2)]
# the final unit is split across both engines so the stream tail drains
# as early as possible
_COPY_PLAN[NUNIT - 1] = [("v", 4, 2), ("a", 6, 2)]

# engine-stream position (1-based) of each copy piece, in emission order
_COPY_POS = {}
_cnt = {"v": 0, "a": 0}
for _u in range(NUNIT):
    for _i, (_e, _s0, _ns) in enumerate(_COPY_PLAN[_u]):
        _cnt[_e] += 1
        _COPY_POS[(_u, _i)] = (_e, _cnt[_e])


def _unit_done_pos(u):
    """Per-engine copy positions after which every slot of unit u is
    drained -> dict engine -> min required count."""
    need = {}
    for i, (e, _s0, _ns) in enumerate(_COPY_PLAN[u]):
        _e, pos = _COPY_POS[(u, i)]
        need[_e] = max(need.get(_e, 0), pos)
    return need


def _group_need(g):
    """Copy positions needed before group g's stage buffer is full."""
    need = {}
    for u in range(UPG * g, UPG * (g + 1)):
        for e, pos in _unit_done_pos(u).items():
            need[e] = max(need.get(e, 0), pos)
    return need


def _build_bass():
    import concourse.mybir as mybir
    from concourse import bacc
    from concourse.bass import ts

    f32 = mybir.dt.float32
    f16 = mybir.dt.float16
    nc = bacc.Bacc(None, target_bir_lowering=False)
    xt = nc.dram_tensor("xt", [PAIR * KR, XC], f16, kind="ExternalInput")
    rhs = nc.dram_tensor("rhs", [PAIR * KR, B_PER_CORE * PAIR * FC], f16,
                         kind="ExternalInput")
    out = nc.dram_tensor("out", [PTS, FC], f16, kind="ExternalOutput")
    out_v = out[:].rearrange("(g j w) c -> g j (w c)", g=GROUPS, j=128, w=TPG)
    out_v4 = out[:].rearrange("(g j w) c -> g j w c", g=GROUPS, j=128, w=TPG)

    from contextlib import ExitStack
    ctx = ExitStack()
    rhs_sb = ctx.enter_context(
        nc.sbuf_tensor("rhs_sb", [PAIR * KR, B_PER_CORE * PAIR * FC], f16))
    xt_sb = ctx.enter_context(nc.sbuf_tensor("xt_sb", [PAIR * KR, XC], f16))
    stage = [ctx.enter_context(
        nc.sbuf_tensor(f"stage{i}", [128, TPG * FC], f16))
        for i in range(N_STAGE)]
    ps = ctx.enter_context(nc.psum_tensor("ps", [128, 8 * SLOT], f32))
    s_rhs = ctx.enter_context(nc.semaphore("s_rhs"))
    s_x = ctx.enter_context(nc.semaphore("s_x"))
    s_pe = ctx.enter_context(nc.semaphore("s_pe"))
    s_cpv = ctx.enter_context(nc.semaphore("s_cpv"))
    s_cpa = ctx.enter_context(nc.semaphore("s_cpa"))
    s_slot = [ctx.enter_context(nc.semaphore(f"s_slot{i}"))
              for i in range(N_STAGE)]

    # ---- input DMAs: xt sits on 8 SBUF partitions = ~2 SDMA engines
    # (~54 GB/s), so it is chunked: a small first chunk gates matmul 0
    # and the rest lands under the running stream.  rhs (23 KB) goes on
    # the ACT ring in parallel. ----
    XA, XB = 8 * 128, 32 * 128        # chunk ends (matmuls 0-7, 8-31)
    nc.sync.dma_start(out=xt_sb[:, :XA], in_=xt[:][:, :XA]).then_inc(s_x, 16)
    nc.scalar.dma_start(out=rhs_sb[:], in_=rhs[:]).then_inc(s_rhs, 16)
    nc.sync.dma_start(out=xt_sb[:, XA:XB], in_=xt[:][:, XA:XB]).then_inc(s_x, 16)
    nc.sync.dma_start(out=xt_sb[:, XB:], in_=xt[:][:, XB:]).then_inc(s_x, 16)

    # out-DMA incs previously issued on each stage slot, for reuse waits
    prior_dmas = {3: 5}

    # ---- PE: 64 back-to-back matmuls ----
    # (The PE HAM clock gate never opens in this environment — a 3.4 us
    # dummy warm-up burst was measured to leave the stream at 1.2 GHz —
    # so the matmuls run at the cold 300 ns/360-col cadence and the copy
    # plan is shaped to never stall the PE.)
    for j in range(NMM):
        lb = j // 16
        u = j // 4
        if j == 0:
            nc.tensor.wait_ge(s_x, 16)
            nc.tensor.wait_ge(s_rhs, 16)
        elif j == 8:
            nc.tensor.wait_ge(s_x, 32)
        elif j == 32:
            nc.tensor.wait_ge(s_x, 48)
        if j % 4 == 0 and u >= 2:
            # psum half reuse: wait for the copies that drained unit u-2
            for e, pos in _unit_done_pos(u - 2).items():
                nc.tensor.wait_ge(s_cpv if e == "v" else s_cpa, pos)
        slot = j % 8
        nc.tensor.matmul(
            ps[:, slot * SLOT: slot * SLOT + PAIR * FC],
            xt_sb[:, ts(j, 128)],
            rhs_sb[:, ts(lb, PAIR * FC)],
            start=True, stop=True,
        ).then_inc(s_pe, 1)

    # ---- DVE/ACT: PSUM -> fp16 stage copies per _COPY_PLAN ----
    def emit_copies(engine_name):
        engine = nc.vector if engine_name == "v" else nc.scalar
        for u in range(NUNIT):
            g = u // UPG
            first_in_group = (u % UPG) == min(
                uu for uu in range(UPG)
                if any(e == engine_name for e, _s, _n in _COPY_PLAN[g * UPG + uu]))
            for i, (e, s0, ns) in enumerate(_COPY_PLAN[u]):
                if e != engine_name:
                    continue
                if g >= N_STAGE and first_in_group and i == min(
                        ii for ii, (ee, _s, _n) in enumerate(_COPY_PLAN[u])
                        if ee == engine_name):
                    engine.wait_ge(s_slot[g % N_STAGE],
                                   16 * prior_dmas[g])
                engine.wait_ge(s_pe, 4 * u + (s0 - (u % 2) * 4) + ns)
                src = (ps[:, s0 * SLOT:(s0 + ns) * SLOT]
                       .rearrange("p (s c) -> p s c", s=ns, c=SLOT)
                       [:, :, :PAIR * FC])
                base = (u % UPG) * 4 * PAIR * FC
                within = s0 - (u % 2) * 4
                dst = (stage[g % N_STAGE]
                       [:, base + within * PAIR * FC:
                        base + (within + ns) * PAIR * FC]
                       .rearrange("p (s c) -> p s c", s=ns, c=PAIR * FC))
                if engine_name == "v":
                    nc.vector.tensor_copy(out=dst, in_=src).then_inc(s_cpv, 1)
                else:
                    nc.scalar.copy(out=dst, in_=src).then_inc(s_cpa, 1)
            # ACT issues group 1's DMA after its copy that completes the
            # group, and the last group's odd units as per-unit DMAs so
            # the stream tail isn't gated on the whole group
            if engine_name == "a" and g == 1 and u == UPG * 2 - 1:
                need = _group_need(g)
                if need.get("v"):
                    nc.scalar.wait_ge(s_cpv, need["v"])
                nc.scalar.dma_start(
                    out=out_v[g], in_=stage[g % N_STAGE][:]
                ).then_inc(s_slot[g % N_STAGE], 16)
            if engine_name == "a" and g == GROUPS - 1 and u % 2 == 1:
                w0 = (u % UPG) * (TPG // UPG)
                w1 = w0 + TPG // UPG
                if u == NUNIT - 1:
                    w0 += TPG // UPG // 2   # ACT half: slots 6-7 only
                nc.scalar.dma_start(
                    out=out_v4[GROUPS - 1][:, w0:w1, :],
                    in_=stage[(GROUPS - 1) % N_STAGE][:, w0 * FC:w1 * FC],
                ).then_inc(s_slot[(GROUPS - 1) % N_STAGE], 16)

    emit_copies("a")
    emit_copies("v")

    # ---- SP: group 0 split DMAs, group 2, and the last group's even
    # units (per-unit, so the stream tail drains as copies land) ----
    g0_waits = []
    for pieces, w0, w1 in (([(0, 0)], 0, 2), ([(0, 1)], 2, 4),
                           ([(0, 2)], 4, 8), ([(1, 0), (1, 1)], 8, 16),
                           ([(2, 1), (3, 1)], 16, TPG)):
        nv = na = 0
        for up in pieces:
            e, pos = _COPY_POS[up]
            if e == "v":
                nv = max(nv, pos)
            else:
                na = max(na, pos)
        g0_waits.append((nv, na, w0, w1))
    for nv, na, w0, w1 in g0_waits:
        if nv:
            nc.sync.wait_ge(s_cpv, nv)
        if na:
            nc.sync.wait_ge(s_cpa, na)
        nc.sync.dma_start(
            out=out_v4[0][:, w0:w1, :],
            in_=stage[0][:, w0 * FC:w1 * FC],
        ).then_inc(s_slot[0], 16)
    g = 2
    need = _group_need(g)
    if need.get("v"):
        nc.sync.wait_ge(s_cpv, need["v"])
    if need.get("a"):
        nc.sync.wait_ge(s_cpa, need["a"])
    nc.sync.dma_start(out=out_v[g], in_=stage[g % N_STAGE][:]).then_inc(
        s_slot[g % N_STAGE], 16)
    for u in range(UPG * (GROUPS - 1), NUNIT):
        if u % 2 == 1 and u != NUNIT - 1:
            continue      # odd units issue on the ACT ring
        w0 = (u % UPG) * (TPG // UPG)
        w1 = w0 + TPG // UPG
        if u == NUNIT - 1:
            w1 -= TPG // UPG // 2   # SP half: the DVE-copied slots 4-5
            nc.sync.wait_ge(s_cpv, _COPY_POS[(u, 0)][1])
        else:
            nc.sync.wait_ge(s_cpv, _unit_done_pos(u)["v"])
        nc.sync.dma_start(
            out=out_v4[GROUPS - 1][:, w0:w1, :],
            in_=stage[(GROUPS - 1) % N_STAGE][:, w0 * FC:w1 * FC],
        ).then_inc(s_slot[(GROUPS - 1) % N_STAGE], 16)

    ctx.close()
    nc.finalize()
    return nc


_NC_CACHE = None
_LAST_RESULTS = None  # BassKernelResults of the most recent run (for profiling)


def kernel(z, mask, initial_grid, W_pe, b_pe, W_clip, b_clip, emb_table,
           W_final, b_final):
    global _NC_CACHE, _LAST_RESULTS
    from concourse import bass_utils

    Q_all, r_all = _precompute(z, W_pe, b_pe, W_clip, b_clip, emb_table,
                               W_final, b_final)
    X = np.ascontiguousarray(np.asarray(initial_grid), dtype=np.float32)

    in_maps = []
    for c in range(NCORES):
        Xc = X[B_PER_CORE * c:B_PER_CORE * (c + 1)].reshape(PTS, NFEATS)
        # point p = g*4096 + j*32 + w lives at tile (g, w), psum partition j
        X4 = Xc.reshape(GROUPS, 128, TPG, NFEATS).transpose(3, 0, 2, 1)
        A = np.empty((GROUPS, TPG, KR, 128), np.float32)
        for k in range(NFEATS):
            A[:, :, k, :] = X4[k]
        A[:, :, NFEATS, :] = 1.0                        # bias row
        # matmul j covers tiles (2*(j%16), 2*(j%16)+1) of group j//16;
        # stationary rows 4a..4a+3 hold tile a of the pair
        xt_host = (A.reshape(GROUPS, TPG // PAIR, PAIR, KR, 128)
                   .transpose(2, 3, 0, 1, 4)
                   .reshape(PAIR * KR, XC)).astype(np.float16)

        rhs_host = np.zeros((PAIR * KR, B_PER_CORE * PAIR * FC), np.float32)
        for lb in range(B_PER_CORE):
            R = np.empty((KR, FC), np.float32)
            R[:NFEATS] = Q_all
            R[NFEATS] = r_all[B_PER_CORE * c + lb]
            for a in range(PAIR):                       # block-diagonal
                rhs_host[KR * a:KR * (a + 1),
                         lb * PAIR * FC + FC * a: lb * PAIR * FC + FC * (a + 1)] = R
        in_maps.append({"xt": np.ascontiguousarray(xt_host),
                        "rhs": rhs_host.astype(np.float16)})

    if _NC_CACHE is None:
        _NC_CACHE = _build_bass()
    res = bass_utils.run_bass_kernel_spmd(
        _NC_CACHE, in_maps, core_ids=list(range(NCORES))
    )
    _LAST_RESULTS = res

    out = np.empty((BS, NJOINTS, NFEATS, NFRAMES), np.float32)
    for c in range(NCORES):
        out[B_PER_CORE * c:B_PER_CORE * (c + 1)] = (
            np.asarray(res.results[c]["out"], dtype=np.float32)
            .reshape(B_PER_CORE, NJOINTS, NFEATS, NFRAMES)
        )
    return out


# revision 31
# speedup vs baseline: 1.1335x; 1.0700x over previous
"""Trainium2 Bass kernel for nn_Decoder_TRANSFORMER_14791867367496.

The reference decoder is affine in the positions: each frame step is
    pos_{t+1} = pos_t @ M + (d_t[b] + g[b,j]),   M = I + W_pe @ W3  (3x3)
(with W_final = [W1; W2; W3] split along its 768 input rows), so the whole
60-step scan has a closed form

    out[b, j, :, t] = X[b, j, :] @ Q_t + r_t[b, :]

where X = initial_grid,
    Q_t = M^t + (W_pe @ W2) @ S_t,          S_t = sum_{k<t} M^k
    r_t[b] = h @ S_t + D_t[b],              D_t = sum_{s=1..t} d_s M^{t-s}
    d_t[b] = (emb_table[t] + z @ W_clip + b_clip) @ W1
    h      = b_pe @ (W2 + W3) + b_final

All of Q/r are tiny (3x3 / per-batch 3-vectors) and are computed on the host
in float64.  The device kernel is then a single affine map per point
([3 feats + bias] -> 180 outputs) and is purely output-bandwidth bound.

The whole device pipeline runs in fp16: the correctness gate is an L2
relative error of 2e-2 and fp16 operands + fp16 output storage land at
~2.9e-4, so the kernel streams the output as fp16 (half the HBM bytes of
f32 — the per-core HBM limit ~358 GB/s is the roofline) and the host
unshard step upcasts to f32.

Device structure (per core: 4 batches = 16384 points = 128 point-tiles):
 - 64 matmuls, each covering a pair of point-tiles ([K=8, 128] stationary
   x [8, 360] block-diagonal rhs -> [128, 360] PSUM).  Sequential MMs at
   one tile position keep the LDWEIGHTS double-buffer path correct (a
   concurrent 4-position row-tiled variant measurably corrupts the
   streaming matmul's weights).  A dense back-to-back MM stream also
   un-throttles the PE HAM clock gate (1.2 -> 2.4 GHz) ~3.4 us in.
 - One [128, 4096] f32 PSUM tensor = all 8 banks; matmul j writes the
   512-col-aligned slot j%8.
 - PSUM->SBUF fp16-converting copies run 1 elem/cycle (PSUM source keeps
   DVE/ACT at 1x mode), so per-instruction fixed cost is amortized with
   wide strided copies: one copy per 4 matmuls reads 4 slots (FD=1440).
   DVE takes even units, ACT odd units; units 0/1 are split finer so the
   output stream starts right after matmul 0.
 - Output: 4 groups x 1.47 MB (per-DMA efficiency ~341 GB/s at this
   size); group 0 goes out as 1/16,1/16,1/8,1/4,1/2.  3 stage buffers
   decouple copies from DMA.  Odd groups issue on the ACT HWDGE ring,
   the rest on SP, so per-DMA setup bubbles overlap.
"""

import numpy as np

BS, NFRAMES, NJOINTS, NFEATS, LATENT, CLIP = 32, 60, 4096, 3, 256, 512
NCORES = 8
B_PER_CORE = BS // NCORES                  # 4
PTS = B_PER_CORE * NJOINTS                 # 16384 points per core
NTILES = PTS // 128                        # 128 point-tiles per core
GROUPS = 4                                 # output DMA groups
TPG = NTILES // GROUPS                     # 32 tiles per group
FC = NFEATS * NFRAMES                      # 180 output columns per point
KR = 4                                     # K rows per tile (3 feats + bias)
PAIR = 2                                   # tiles fused per matmul
NMM = NTILES // PAIR                       # 64 matmuls per core
NUNIT = NMM // 4                           # 16 copy units (4 MMs each)
XC = NMM * 128                             # xt columns (8192)
SLOT = 512                                 # psum cols per matmul slot (bank)
UPG = NUNIT // GROUPS                      # 4 copy units per group


def _precompute(z, W_pe, b_pe, W_clip, b_clip, emb_table, W_final, b_final):
    """Host-side f64 computation of the closed-form coefficients.

    Returns Q_all [3, 180] and r_all [32, 180], column layout c = f*60 + t
    (matching the [.., 3, 60] innermost layout of the output)."""
    f64 = np.float64
    W_pe64 = np.asarray(W_pe, f64)
    W_fin = np.asarray(W_final, f64)
    W1, W2, W3 = W_fin[:LATENT], W_fin[LATENT:2 * LATENT], W_fin[2 * LATENT:]
    M = np.eye(3) + W_pe64 @ W3
    Gm = W_pe64 @ W2
    b_pe64 = np.asarray(b_pe, f64)
    h = b_pe64 @ W2 + b_pe64 @ W3 + np.asarray(b_final, f64)
    z_proj = np.asarray(z, f64) @ np.asarray(W_clip, f64) + np.asarray(b_clip, f64)
    d = (np.asarray(emb_table, f64)[None, :, :] + z_proj[:, None, :]) @ W1  # [32,60,3]

    Q = np.zeros((NFRAMES, 3, 3))
    R = np.zeros((NFRAMES, BS, 3))
    Q[0] = np.eye(3)
    Mt = np.eye(3)
    S = np.zeros((3, 3))
    D = np.zeros((BS, 3))
    for t in range(1, NFRAMES):
        S = S + Mt
        Mt = Mt @ M
        D = D @ M + d[:, t, :]
        Q[t] = Mt + Gm @ S
        R[t] = h @ S + D
    Q_all = Q.transpose(1, 2, 0).reshape(3, FC)     # [k, f*60+t]
    r_all = R.transpose(1, 2, 0).reshape(BS, FC)    # [b, f*60+t]
    return Q_all.astype(np.float32), r_all.astype(np.float32)


N_STAGE = 3   # stage buffers

# copy plan: unit u (= matmuls 4u..4u+3, psum slots (u%2)*4..+3) ->
# list of (engine, first_slot, n_slots) pieces.  Unit 0 is split fine so
# the first output DMAs can start early; from unit 1 on, two FD=720
# pieces per unit (DVE even units, ACT odd) — the first piece starts two
# matmuls early, which removes the PE psum-wrap stall entirely.
_COPY_PLAN = {0: [("v", 0, 1), ("a", 1, 1), ("v", 2, 2)]}
for _u in range(1, NUNIT):
    _e = "v" if _u % 2 == 0 else "a"
    _p = (_u % 2) * 4
    _COPY_PLAN[_u] = [(_e, _p, 2), (_e, _p + 2, 2)]
# the final unit is split across both engines so the stream tail drains
# as early as possible
_COPY_PLAN[NUNIT - 1] = [("v", 4, 2), ("a", 6, 2)]

# engine-stream position (1-based) of each copy piece, in emission order
_COPY_POS = {}
_cnt = {"v": 0, "a": 0}
for _u in range(NUNIT):
    for _i, (_e, _s0, _ns) in enumerate(_COPY_PLAN[_u]):
        _cnt[_e] += 1
        _COPY_POS[(_u, _i)] = (_e, _cnt[_e])


def _unit_done_pos(u):
    """Per-engine copy positions after which every slot of unit u is
    drained -> dict engine -> min required count."""
    need = {}
    for i, (e, _s0, _ns) in enumerate(_COPY_PLAN[u]):
        _e, pos = _COPY_POS[(u, i)]
        need[_e] = max(need.get(_e, 0), pos)
    return need


def _group_need(g):
    """Copy positions needed before group g's stage buffer is full."""
    need = {}
    for u in range(UPG * g, UPG * (g + 1)):
        for e, pos in _unit_done_pos(u).items():
            need[e] = max(need.get(e, 0), pos)
    return need


def _build_bass():
    import concourse.mybir as mybir
    from concourse import bacc
    from concourse.bass import ts

    f32 = mybir.dt.float32
    f16 = mybir.dt.float16
    nc = bacc.Bacc(None, target_bir_lowering=False)
    xt = nc.dram_tensor("xt", [128, XC // 4], f16, kind="ExternalInput")
    rhs = nc.dram_tensor("rhs", [128, B_PER_CORE * PAIR * FC], f16,
                         kind="ExternalInput")
    out = nc.dram_tensor("out", [PTS, FC], f16, kind="ExternalOutput")
    out_v = out[:].rearrange("(g j w) c -> g j (w c)", g=GROUPS, j=128, w=TPG)
    out_v4 = out[:].rearrange("(g j w) c -> g j w c", g=GROUPS, j=128, w=TPG)

    from contextlib import ExitStack
    ctx = ExitStack()
    rhs_sb = ctx.enter_context(
        nc.sbuf_tensor("rhs_sb", [128, B_PER_CORE * PAIR * FC], f16))
    xt_sb = ctx.enter_context(nc.sbuf_tensor("xt_sb", [128, XC // 4], f16))
    stage = [ctx.enter_context(
        nc.sbuf_tensor(f"stage{i}", [128, TPG * FC], f16))
        for i in range(N_STAGE)]
    ps = ctx.enter_context(nc.psum_tensor("ps", [128, 8 * SLOT], f32))
    s_rhs = ctx.enter_context(nc.semaphore("s_rhs"))
    s_x = ctx.enter_context(nc.semaphore("s_x"))
    s_pe = ctx.enter_context(nc.semaphore("s_pe"))
    s_cpv = ctx.enter_context(nc.semaphore("s_cpv"))
    s_cpa = ctx.enter_context(nc.semaphore("s_cpa"))
    s_slot = [ctx.enter_context(nc.semaphore(f"s_slot{i}"))
              for i in range(N_STAGE)]

    # ---- input DMAs (strip data pre-padded across all 128 partitions =
    # full SDMA bandwidth): a small first chunk gates matmul 0 and the
    # rest lands under the running stream.  rhs goes on the ACT ring. ----
    XA, XB = 2 * 128, 8 * 128         # chunk ends (matmuls 0-7, 8-31)
    RA = PAIR * FC                     # rhs chunk for lb=0 (matmuls 0-15)
    nc.sync.dma_start(out=xt_sb[:, :XA], in_=xt[:][:, :XA]).then_inc(s_x, 16)
    nc.scalar.dma_start(out=rhs_sb[:, :RA],
                        in_=rhs[:][:, :RA]).then_inc(s_rhs, 16)
    nc.sync.dma_start(out=xt_sb[:, XA:XB], in_=xt[:][:, XA:XB]).then_inc(s_x, 16)
    nc.sync.dma_start(out=xt_sb[:, XB:], in_=xt[:][:, XB:]).then_inc(s_x, 16)
    nc.scalar.dma_start(out=rhs_sb[:, RA:],
                        in_=rhs[:][:, RA:]).then_inc(s_rhs, 16)

    # out-DMA incs previously issued on each stage slot, for reuse waits
    prior_dmas = {3: 5}

    # ---- PE: 64 back-to-back matmuls ----
    # (The PE HAM clock gate never opens in this environment — a 3.4 us
    # dummy warm-up burst was measured to leave the stream at 1.2 GHz —
    # so the matmuls run at the cold 300 ns/360-col cadence and the copy
    # plan is shaped to never stall the PE.)
    for j in range(NMM):
        lb = j // 16
        u = j // 4
        if j == 0:
            nc.tensor.wait_ge(s_x, 16)
            nc.tensor.wait_ge(s_rhs, 16)
        elif j == 8:
            nc.tensor.wait_ge(s_x, 32)
        elif j == 16:
            nc.tensor.wait_ge(s_rhs, 32)
        elif j == 32:
            nc.tensor.wait_ge(s_x, 48)
        if j % 4 == 0 and u >= 2:
            # psum half reuse: wait for the copies that drained unit u-2
            for e, pos in _unit_done_pos(u - 2).items():
                nc.tensor.wait_ge(s_cpv if e == "v" else s_cpa, pos)
        slot = j % 8
        # duo d=j//2 member m=j%2 runs in SBUF strip 32*((d%2)+2m); duos
        # alternate strip pairs {0,64}/{32,96} so a duo's LDWEIGHTS never
        # targets a strip whose previous matmul is still streaming
        d, m = j // 2, j % 2
        P = 32 * ((d % 2) + 2 * m)
        nc.tensor.matmul(
            ps[:, slot * SLOT: slot * SLOT + PAIR * FC],
            xt_sb[P:P + PAIR * KR, ts(j // 4, 128)],
            rhs_sb[P:P + PAIR * KR, ts(lb, PAIR * FC)],
            start=True, stop=True, tile_position=(P, 0),
        ).then_inc(s_pe, 1)

    # ---- DVE/ACT: PSUM -> fp16 stage copies per _COPY_PLAN ----
    def emit_copies(engine_name):
        engine = nc.vector if engine_name == "v" else nc.scalar
        for u in range(NUNIT):
            g = u // UPG
            first_in_group = (u % UPG) == min(
                uu for uu in range(UPG)
                if any(e == engine_name for e, _s, _n in _COPY_PLAN[g * UPG + uu]))
            for i, (e, s0, ns) in enumerate(_COPY_PLAN[u]):
                if e != engine_name:
                    continue
                if g >= N_STAGE and first_in_group and i == min(
                        ii for ii, (ee, _s, _n) in enumerate(_COPY_PLAN[u])
                        if ee == engine_name):
                    engine.wait_ge(s_slot[g % N_STAGE],
                                   16 * prior_dmas[g])
                engine.wait_ge(s_pe, 4 * u + (s0 - (u % 2) * 4) + ns)
                src = (ps[:, s0 * SLOT:(s0 + ns) * SLOT]
                       .rearrange("p (s c) -> p s c", s=ns, c=SLOT)
                       [:, :, :PAIR * FC])
                base = (u % UPG) * 4 * PAIR * FC
                within = s0 - (u % 2) * 4
                dst = (stage[g % N_STAGE]
                       [:, base + within * PAIR * FC:
                        base + (within + ns) * PAIR * FC]
                       .rearrange("p (s c) -> p s c", s=ns, c=PAIR * FC))
                if engine_name == "v":
                    nc.vector.tensor_copy(out=dst, in_=src).then_inc(s_cpv, 1)
                else:
                    nc.scalar.copy(out=dst, in_=src).then_inc(s_cpa, 1)
            # ACT issues group 1's DMA after its copy that completes the
            # group, and the last group's odd units as per-unit DMAs so
            # the stream tail isn't gated on the whole group
            if engine_name == "a" and g == 1 and u == UPG * 2 - 1:
                need = _group_need(g)
                if need.get("v"):
                    nc.scalar.wait_ge(s_cpv, need["v"])
                nc.scalar.dma_start(
                    out=out_v[g], in_=stage[g % N_STAGE][:]
                ).then_inc(s_slot[g % N_STAGE], 16)
            if engine_name == "a" and g == GROUPS - 1 and u % 2 == 1:
                w0 = (u % UPG) * (TPG // UPG)
                w1 = w0 + TPG // UPG
                if u == NUNIT - 1:
                    w0 += TPG // UPG // 2   # ACT half: slots 6-7 only
                nc.scalar.dma_start(
                    out=out_v4[GROUPS - 1][:, w0:w1, :],
                    in_=stage[(GROUPS - 1) % N_STAGE][:, w0 * FC:w1 * FC],
                ).then_inc(s_slot[(GROUPS - 1) % N_STAGE], 16)

    emit_copies("a")
    emit_copies("v")

    # ---- SP: group 0 split DMAs, group 2, and the last group's even
    # units (per-unit, so the stream tail drains as copies land) ----
    g0_waits = []
    for pieces, w0, w1 in (([(0, 0)], 0, 2), ([(0, 1)], 2, 4),
                           ([(0, 2)], 4, 8), ([(1, 0), (1, 1)], 8, 16),
                           ([(2, 1), (3, 1)], 16, TPG)):
        nv = na = 0
        for up in pieces:
            e, pos = _COPY_POS[up]
            if e == "v":
                nv = max(nv, pos)
            else:
                na = max(na, pos)
        g0_waits.append((nv, na, w0, w1))
    for nv, na, w0, w1 in g0_waits:
        if nv:
            nc.sync.wait_ge(s_cpv, nv)
        if na:
            nc.sync.wait_ge(s_cpa, na)
        nc.sync.dma_start(
            out=out_v4[0][:, w0:w1, :],
            in_=stage[0][:, w0 * FC:w1 * FC],
        ).then_inc(s_slot[0], 16)
    g = 2
    need = _group_need(g)
    if need.get("v"):
        nc.sync.wait_ge(s_cpv, need["v"])
    if need.get("a"):
        nc.sync.wait_ge(s_cpa, need["a"])
    nc.sync.dma_start(out=out_v[g], in_=stage[g % N_STAGE][:]).then_inc(
        s_slot[g % N_STAGE], 16)
    for u in range(UPG * (GROUPS - 1), NUNIT):
        if u % 2 == 1 and u != NUNIT - 1:
            continue      # odd units issue on the ACT ring
        w0 = (u % UPG) * (TPG // UPG)
        w1 = w0 + TPG // UPG
        if u == NUNIT - 1:
            w1 -= TPG // UPG // 2   # SP half: the DVE-copied slots 4-5
            nc.sync.wait_ge(s_cpv, _COPY_POS[(u, 0)][1])
        else:
            nc.sync.wait_ge(s_cpv, _unit_done_pos(u)["v"])
        nc.sync.dma_start(
            out=out_v4[GROUPS - 1][:, w0:w1, :],
            in_=stage[(GROUPS - 1) % N_STAGE][:, w0 * FC:w1 * FC],
        ).then_inc(s_slot[(GROUPS - 1) % N_STAGE], 16)

    ctx.close()
    nc.finalize()
    return nc


_NC_CACHE = None
_LAST_RESULTS = None  # BassKernelResults of the most recent run (for profiling)


def kernel(z, mask, initial_grid, W_pe, b_pe, W_clip, b_clip, emb_table,
           W_final, b_final):
    global _NC_CACHE, _LAST_RESULTS
    from concourse import bass_utils

    Q_all, r_all = _precompute(z, W_pe, b_pe, W_clip, b_clip, emb_table,
                               W_final, b_final)
    X = np.ascontiguousarray(np.asarray(initial_grid), dtype=np.float32)

    in_maps = []
    for c in range(NCORES):
        Xc = X[B_PER_CORE * c:B_PER_CORE * (c + 1)].reshape(PTS, NFEATS)
        # point p = g*4096 + j*32 + w lives at tile (g, w), psum partition j
        X4 = Xc.reshape(GROUPS, 128, TPG, NFEATS).transpose(3, 0, 2, 1)
        A = np.empty((GROUPS, TPG, KR, 128), np.float32)
        for k in range(NFEATS):
            A[:, :, k, :] = X4[k]
        A[:, :, NFEATS, :] = 1.0                        # bias row
        # matmul j covers tiles (2*(j%16), 2*(j%16)+1) of group j//16;
        # stationary rows 4a..4a+3 hold tile a of the pair
        xt_flat = (A.reshape(GROUPS, TPG // PAIR, PAIR, KR, 128)
                   .transpose(2, 3, 0, 1, 4)
                   .reshape(PAIR * KR, NMM, 128))
        # matmul j = duo d=j//2 member m=j%2 -> strip 32*((d%2)+2m),
        # strip-local column block j//4
        xt_host = np.zeros((128, XC // 4), np.float16)
        for j in range(NMM):
            d, m = j // 2, j % 2
            P = 32 * ((d % 2) + 2 * m)
            xt_host[P:P + PAIR * KR, (j // 4) * 128:(j // 4 + 1) * 128] = \
                xt_flat[:, j, :]

        rhs_blk = np.zeros((PAIR * KR, B_PER_CORE * PAIR * FC), np.float32)
        for lb in range(B_PER_CORE):
            R = np.empty((KR, FC), np.float32)
            R[:NFEATS] = Q_all
            R[NFEATS] = r_all[B_PER_CORE * c + lb]
            for a in range(PAIR):                       # block-diagonal
                rhs_blk[KR * a:KR * (a + 1),
                        lb * PAIR * FC + FC * a: lb * PAIR * FC + FC * (a + 1)] = R
        rhs_host = np.zeros((128, B_PER_CORE * PAIR * FC), np.float16)
        for sp in range(4):
            rhs_host[32 * sp:32 * sp + PAIR * KR] = rhs_blk.astype(np.float16)
        in_maps.append({"xt": np.ascontiguousarray(xt_host),
                        "rhs": np.ascontiguousarray(rhs_host)})

    if _NC_CACHE is None:
        _NC_CACHE = _build_bass()
    res = bass_utils.run_bass_kernel_spmd(
        _NC_CACHE, in_maps, core_ids=list(range(NCORES))
    )
    _LAST_RESULTS = res

    out = np.empty((BS, NJOINTS, NFEATS, NFRAMES), np.float32)
    for c in range(NCORES):
        out[B_PER_CORE * c:B_PER_CORE * (c + 1)] = (
            np.asarray(res.results[c]["out"], dtype=np.float32)
            .reshape(B_PER_CORE, NJOINTS, NFEATS, NFRAMES)
        )
    return out
